# revision 53
# baseline (speedup 1.0000x reference)
"""Trainium2 Bass kernel: GQA attention layer with RoPE + int8 quant-dequant KV.

Tensor-parallel over heads across 8 NeuronCores: core c owns q-heads
[4c, 4c+4) and kv-head c.  Each core computes its partial output
y_c = attn_out_local @ wo_local.T (bf16); the host sums the 8 partials.

Per-core design (all [feature, token] with feature on SBUF partitions):
- Weights persist in SBUF (loaded once, MB-scale batched DMAs: per-DMA
  issue overhead otherwise starves the PE); hs streams per query tile in
  quarter-tile chunks.
- KV int8 quant-dequant runs directly in [feat, tok] layout using the
  Pool engine's partition absmax all-reduce (no transposes/copies), RNE
  via the fp32 magic-constant trick.
- Softmax denominator: exp tiles are pair-summed on DVE, partition-summed
  on Pool (replaces PE ones-matmuls); reciprocal broadcast via Pool
  partition_broadcast (no DRAM round trip).
- Diagonal causal blocks restrict scores/exp/AV to live columns; a single
  [128,128] triangular mask covers the one partially-masked band.
- The in-order PE never waits on other engines: the s-matmul pipeline
  runs PIPE steps ahead of exp/AV tails across head boundaries; each
  tile's wo GEMM and the next tile's k/v projections are deferred and
  interleaved (alternating) into the attention loop as PE filler, which
  also keeps the PE clock ramped (the PE DVFS resets on every idle gap);
  tile 0 interleaves q projections per DMA quarter, later tiles close
  each q head's accumulation early (h-outer) so its RoPE chain overlaps
  the rest; y is written bf16 on the ACT engine, row-batched.
"""
import math
import numpy as np
from contextlib import ExitStack

import concourse.bass as bass
import concourse.bacc as bacc
import concourse.mybir as mybir
import concourse.bass_isa as bass_isa
import concourse.tile as tile
from concourse.bass_utils import run_bass_kernel_spmd
from concourse.masks import make_identity

F32 = mybir.dt.float32
BF16 = mybir.dt.bfloat16
AF = mybir.ActivationFunctionType
ALU = mybir.AluOpType
AX = mybir.AxisListType
ROP = bass_isa.ReduceOp

MAGIC = 1.5 * 2.0**23  # fp32 RNE integer-rounding magic constant
NCORES = 8
REGIONS = []  # (label, first_id) markers filled during build, for profiling


def build_nc(S=2048, D=4096, HL=4, QT=512, nreps=1):
    """Build the per-core Bass graph. HL = local q heads (1 local kv head)."""
    MMDT = BF16
    DT = D // 128    # contraction tiles for projections
    NQ = S // QT     # query tiles
    DB = QT // 128   # 128-blocks per query tile
    KB = S // 128    # total k blocks
    NDC = D // 512   # wo output column tiles

    nc = bacc.Bacc("TRN2")
    hsT = nc.declare_dram_parameter("hsT", [D, S], MMDT, isOutput=False)
    wqT = nc.declare_dram_parameter("wqT", [D, HL * 128], MMDT, isOutput=False)
    wkT = nc.declare_dram_parameter("wkT", [D, 128], MMDT, isOutput=False)
    wvT = nc.declare_dram_parameter("wvT", [D, 128], MMDT, isOutput=False)
    woT = nc.declare_dram_parameter("woT", [HL * 128, D], BF16, isOutput=False)
    cosT = nc.declare_dram_parameter("cosT", [128, S], F32, isOutput=False)
    sinT = nc.declare_dram_parameter("sinT", [128, S], F32, isOutput=False)
    mks = nc.declare_dram_parameter("mks", [128, 128], BF16, isOutput=False)
    rotT = nc.declare_dram_parameter("rotT", [128, 128], MMDT, isOutput=False)
    y = nc.declare_dram_parameter("y", [S, D], BF16, isOutput=True)

    with tile.TileContext(nc) as tc, ExitStack() as ctx:
        const = ctx.enter_context(tc.tile_pool(name="const", bufs=1))
        persist = ctx.enter_context(tc.tile_pool(name="persist", bufs=1))
        hsq_pool = ctx.enter_context(tc.tile_pool(name="hsq", bufs=5))
        work = ctx.enter_context(tc.tile_pool(name="work", bufs=4))
        raws = ctx.enter_context(tc.tile_pool(name="raws", bufs=3))
        expp = ctx.enter_context(tc.tile_pool(name="expp", bufs=4))
        denp = ctx.enter_context(tc.tile_pool(name="denp", bufs=3))
        qpool = ctx.enter_context(tc.tile_pool(name="qpool", bufs=HL + 1))
        apool = ctx.enter_context(tc.tile_pool(name="apool", bufs=2 * HL + 1))
        ypool = ctx.enter_context(tc.tile_pool(name="ypool", bufs=2))
        vqp = ctx.enter_context(tc.tile_pool(name="vqp", bufs=2))
        rowp = ctx.enter_context(tc.tile_pool(name="rowp", bufs=3))
        pbig = ctx.enter_context(tc.tile_pool(name="pbig", bufs=7, space="PSUM"))
        ptr = ctx.enter_context(tc.tile_pool(name="ptr", bufs=1, space="PSUM"))

        rot_sb = const.tile([128, 128], MMDT, name="rot", tag="rot")
        ident = const.tile([128, 128], MMDT, name="ident", tag="ident")
        make_identity(nc, ident[:])

        wq_sb = persist.tile([128, DT, HL * 128], MMDT, name="wq", tag="wq")
        wk_sb = persist.tile([128, DT, 128], MMDT, name="wk", tag="wk")
        wv_sb = persist.tile([128, DT, 128], MMDT, name="wv", tag="wv")
        cos_sb = const.tile([128, S], F32, name="cos", tag="cos")
        sin_sb = const.tile([128, S], F32, name="sin", tag="sin")
        mks_sb = []
        woT_sb = []

        kT_all = persist.tile([128, S], MMDT, name="kT", tag="kT")
        v_nat = persist.tile([128, KB, 128], MMDT, name="vnat", tag="vnat")
        # diagonal-block exp tiles, double-buffered per r: columns < 128r
        # are zeroed once here and never written again
        e_diag = {}
        for r in range(1, DB):
            t = persist.tile([128, QT], MMDT, name=f"ed{r}", tag=f"ed{r}")
            nc.vector.memset(t[:, :r * 128], 0.0)
            e_diag[r] = t

        def qd_T(x_ap, out_ap):
            """int8 quant-dequant of a [feat(part), tok(free)] tile.

            absmax over the feat (partition) axis per token via Pool
            all-reduce (result broadcast to all partitions), symmetric
            127-step grid, RNE via the fp32 magic trick.
            """
            fr = x_ap.shape[1]
            amax = denp.tile([128, QT], F32, name="den", tag="den")
            nc.gpsimd.partition_all_reduce(amax[:, :fr], x_ap, channels=128,
                                           reduce_op=ROP.absmax)
            scl = work.tile([128, QT], F32, name="work", tag="work")
            nc.vector.tensor_scalar(out=scl[:, :fr], in0=amax[:, :fr],
                                    scalar1=1.0 / 127.0, scalar2=1e-8,
                                    op0=ALU.mult, op1=ALU.max)
            inv = work.tile([128, QT], F32, name="work", tag="work")
            nc.vector.reciprocal(inv[:, :fr], scl[:, :fr])
            xs = work.tile([128, QT], F32, name="work", tag="work")
            nc.vector.tensor_tensor(out=xs[:, :fr], in0=x_ap, in1=inv[:, :fr],
                                    op=ALU.mult)
            nc.vector.tensor_scalar(out=xs[:, :fr], in0=xs[:, :fr],
                                    scalar1=MAGIC, scalar2=MAGIC,
                                    op0=ALU.add, op1=ALU.subtract)
            nc.vector.tensor_tensor(out=out_ap, in0=xs[:, :fr],
                                    in1=scl[:, :fr], op=ALU.mult)

        def rope(psum_in, cos_sl, sin_sl, out_ap):
            """RoPE in [feat, tok] layout; rotate-half via permutation matmul."""
            raw = raws.tile([128, QT], MMDT, name="raw", tag="raw")
            nc.scalar.copy(raw[:], psum_in[:])
            rope_tail(raw, cos_sl, sin_sl, out_ap)

        def rope_tail(raw, cos_sl, sin_sl, out_ap):
            rot_ps = pbig.tile([128, QT], F32, name="big", tag="big")
            nc.tensor.matmul(rot_ps[:], rot_sb[:], raw[:], start=True,
                             stop=True)
            tmp = work.tile([128, QT], F32, name="work", tag="work")
            nc.vector.tensor_tensor(out=tmp[:], in0=raw[:], in1=cos_sl,
                                    op=ALU.mult)
            t2 = work.tile([128, QT], F32, name="work", tag="work")
            nc.vector.tensor_tensor(out=t2[:], in0=rot_ps[:], in1=sin_sl,
                                    op=ALU.mult)
            nc.vector.tensor_tensor(out=out_ap, in0=tmp[:], in1=t2[:],
                                    op=ALU.add)

        prev_wo = []  # deferred wo-pair emitters for the previous tile

        prev_wo = []  # deferred wo-pair emitters for the previous tile

        for rep_I in range(nreps * NQ):
            I = rep_I % NQ
            qsl = slice(I * QT, (I + 1) * QT)
            cos_sl = cos_sb[:, qsl]
            sin_sl = sin_sb[:, qsl]

            # ---- k/v projections first: their RoPE/quant chain then
            # overlaps the q projections below ----
            pk = pbig.tile([128, QT], F32, name="big", tag="big")
            pv = pbig.tile([128, QT], F32, name="big", tag="big")
            hs_tiles = []
            for d in range(DT):
                dsl = slice(d * 128, (d + 1) * 128)
                if I == 0:
                    nc.sync.dma_start(out=wk_sb[:, d * 128:(d + 1) * 128],
                                      in_=wkT[dsl, :])
                    nc.sync.dma_start(out=wv_sb[:, d * 128:(d + 1) * 128],
                                      in_=wvT[dsl, :])
                    nc.sync.dma_start(
                        out=wq_sb[:, d * HL * 128:(d + 1) * HL * 128],
                        in_=wqT[dsl, :])
                hs_t = hs_pool.tile([128, QT], MMDT, name="hs", tag="hs")
                nc.sync.dma_start(out=hs_t[:], in_=hsT[dsl, qsl])
                hs_tiles.append(hs_t)
                first, last = d == 0, d == DT - 1
                nc.tensor.matmul(pk[:], wk_sb[:, d * 128:(d + 1) * 128],
                                 hs_t[:], start=first, stop=last)
                nc.tensor.matmul(pv[:], wv_sb[:, d * 128:(d + 1) * 128],
                                 hs_t[:], start=first, stop=last)

            if I == 0:
                nc.sync.dma_start(out=cos_sb[:], in_=cosT[:])
                nc.sync.dma_start(out=sin_sb[:], in_=sinT[:])
                m = const.tile([128, 128], BF16, name="mk", tag="mk")
                nc.sync.dma_start(out=m[:], in_=mks[:])
                mks_sb.append(m)

            # ---- RoPE k + quant-dequant straight into kT_all ----
            krope = work.tile([128, QT], F32, name="work", tag="work")
            rope(pk, cos_sl, sin_sl, krope[:])
            qd_T(krope[:], kT_all[:, qsl])

            # ---- v: quant-dequant in [feat, tok] (transposes are emitted
            # after the q projections so the in-order PE never blocks on
            # this DVE chain) ----
            vraw = work.tile([128, QT], F32, name="work", tag="work")
            nc.scalar.copy(vraw[:], pv[:])
            vq = vqp.tile([128, QT], MMDT, name="vq", tag="vq")
            qd_T(vraw[:], vq[:])

            # ---- q projections (overlap the k/v chain above) ----
            pq = [pbig.tile([128, QT], F32, name="big", tag="big")
                  for _ in range(HL)]
            for d in range(DT):
                first, last = d == 0, d == DT - 1
                for h in range(HL):
                    nc.tensor.matmul(
                        pq[h][:],
                        wq_sb[:, d, h * 128:(h + 1) * 128],
                        hqs[d // CQ][:, d % CQ, :], start=first, stop=last)

            if rep_I == 0:
                for hb in range(HL):
                    w = persist.tile([128, D], BF16, name=f"wo{hb}",
                                     tag=f"wo{hb}")
                    nc.sync.dma_start(out=w[:],
                                      in_=woT[hb * 128:(hb + 1) * 128, :])
                    woT_sb.append(w)

            if rep_I + 1 < nreps * NQ:
                prepare_kvproj(rep_I + 1)
            kv_next = kv_state.get(rep_I + 1, [None, None, None, []])[3]

            # ---- RoPE q ----
            qts = []
            for h in range(HL):
                qt_t = qpool.tile([128, QT], MMDT, name="qt", tag="qt")
                rope(pq[h], cos_sl, sin_sl, qt_t[:])
                qts.append(qt_t)

            # ---- v transpose into natural layout (PE) ----
            for t in range(DB):
                t_sl = slice(t * 128, (t + 1) * 128)
                tr_ps = ptr.tile([128, 128], MMDT, name="tr", tag="tr")
                nc.tensor.transpose(tr_ps[:], vq[:, t_sl], ident[:])
                nc.scalar.copy(v_nat[:, I * DB + t, :], tr_ps[:])

            # ---- attention (causal, unnormalized exp + Pool col-sums).
            # Diagonal blocks r only have live columns q >= 128r: scores
            # and exp are restricted to them, the e prefix is zeroed, and
            # only the [128r, 128r+128) column band needs the triangular
            # mask. ----
            # ---- attention, software-pipelined ACROSS heads: s matmuls
            # run PIPE steps ahead of their exp/AV tails so the in-order
            # PE never drains at head boundaries ----
            ats = {}
            nkb = (I + 1) * DB
            PIPE = 3
            pend = []
            out_pss = {}

            def stage_s(h, j):
                if j == 0:
                    out_pss[h] = pbig.tile([128, QT], F32, name="big",
                                           tag="big")
                r = j - I * DB
                c0 = max(r, 0) * 128  # first live column
                s_ps = pbig.tile([128, QT], F32, name="big", tag="big")
                nc.tensor.matmul(s_ps[:, c0:],
                                 kT_all[:, j * 128:(j + 1) * 128],
                                 qts[h][:, c0:], start=True, stop=True)
                pend.append((h, j, r, c0, s_ps))

            eprev = {}  # h -> pending e tile awaiting its pair
            den_accs = {}

            def den_flush(h, e_sb, force):
                # pair e tiles on DVE; Pool partition-sums once per pair
                if h not in den_accs:
                    den_accs[h] = rowp.tile([1, QT], F32, name="dac",
                                            tag="dac")
                den_acc = den_accs[h][:]
                if h in eprev and e_sb is not None:
                    ep = expp.tile([128, QT], MMDT, name="exp", tag="exp")
                    nc.vector.tensor_tensor(out=ep[:], in0=eprev.pop(h)[:],
                                            in1=e_sb[:], op=ALU.add)
                elif e_sb is not None:
                    eprev[h] = e_sb
                    if not force:
                        return
                    ep = eprev.pop(h)
                else:
                    return
                den_bc = denp.tile([128, QT], F32, name="den", tag="den")
                nc.gpsimd.partition_all_reduce(den_bc[:], ep[:],
                                               channels=128,
                                               reduce_op=ROP.add)
                if (h, "init") not in eprev:
                    eprev[(h, "init")] = True
                    nc.vector.tensor_copy(den_acc, den_bc[0:1, :])
                else:
                    nc.vector.tensor_tensor(out=den_acc, in0=den_acc,
                                            in1=den_bc[0:1, :], op=ALU.add)

            def stage_tail():
                h, j, r, c0, s_ps = pend.pop(0)
                out_ps = out_pss[h]
                if r >= 1:
                    e_sb = e_diag[r]
                else:
                    e_sb = expp.tile([128, QT], MMDT, name="exp", tag="exp")
                nc.scalar.activation(e_sb[:, c0:], s_ps[:, c0:], AF.Exp,
                                     bias=0.0, scale=1.0 / math.sqrt(128.0))
                if r >= 0:
                    nc.vector.tensor_tensor(
                        out=e_sb[:, c0:c0 + 128],
                        in0=e_sb[:, c0:c0 + 128],
                        in1=mks_sb[0][:], op=ALU.mult)
                first, last = j == 0, j == nkb - 1
                nc.tensor.matmul(out_ps[:, c0:], v_nat[:, j, :],
                                 e_sb[:, c0:], start=first, stop=last,
                                 skip_group_check=True)
                den_flush(h, e_sb, force=last)
                if last:
                    eprev.pop((h, "init"), None)
                    rec = den_accs.pop(h)[:]
                    nc.vector.reciprocal(rec, rec)
                    rec_b = denp.tile([128, QT], F32, name="den", tag="den")
                    nc.gpsimd.partition_broadcast(rec_b[:], rec)
                    a_t = apool.tile([128, QT], BF16, name="at", tag="at")
                    nc.vector.tensor_tensor(out=a_t[:], in0=out_ps[:],
                                            in1=rec_b[:], op=ALU.mult)
                    ats[h] = a_t

            total_slots = HL * nkb
            n_fill = len(prev_wo) + len(kv_next)
            slot = 0
            emitted = 0
            for h in range(HL):
                for j in range(nkb):
                    stage_s(h, j)
                    if h == 0 and j == min(2, nkb - 1):
                        emit_vtr()
                    if len(pend) > PIPE:
                        stage_tail()
                    slot += 1
                    while emitted < (slot * n_fill) // total_slots:
                        if kv_next and (emitted % 2 or not prev_wo):
                            kv_next.pop(0)()
                        else:
                            (prev_wo or kv_next).pop(0)()
                        emitted += 1
            while pend:
                stage_tail()
            while prev_wo:
                prev_wo.pop(0)()

            # ---- wo partial: y[tok, dout] += aT.T @ woT.  Deferred: the
            # pairs are emitted interleaved into the NEXT tile's attention
            # loop so the PE fills its exp-wait gaps with wo matmuls ----
            mark(f"wo{I}")

            def make_wo_pair(I_src, t, dc, ats_src, y_rows_src):
                def head(nhb):
                    y_ps = pbig.tile([128, 512], F32, name="big", tag="big")
                    for hb in range(nhb):
                        nc.tensor.matmul(
                            y_ps[:], ats_src[hb][:, t * 128:(t + 1) * 128],
                            woT_sb[hb][:, dc * 512:(dc + 1) * 512],
                            start=(hb == 0), stop=(hb == HL - 1))
                    return y_ps

                def finish(y_ps):
                    for hb in range(HL - 1, HL):
                        nc.tensor.matmul(
                            y_ps[:], ats_src[hb][:, t * 128:(t + 1) * 128],
                            woT_sb[hb][:, dc * 512:(dc + 1) * 512],
                            start=False, stop=True)
                    if t not in y_rows_src:
                        y_rows_src[t] = ypool.tile([128, D], BF16, name="y",
                                                   tag="y")
                    dst = y_rows_src[t][:, dc * 512:(dc + 1) * 512]
                    nc.scalar.copy(dst, y_ps[:])
                    if dc == NDC - 1:
                        nc.sync.dma_start(
                            out=y[I_src * QT + t * 128:
                                  I_src * QT + (t + 1) * 128, :],
                            in_=y_rows_src.pop(t)[:])

                def go():
                    finish(head(HL - 1))
                return go, head, finish

            y_rows = {}
            prev_wo = [make_wo_pair(I, t, dc, ats, y_rows)[0]
                       for t in range(DB) for dc in range(NDC)]
            prev_wo_parts = [make_wo_pair(I, t, dc, ats, y_rows)[1:]
                             for t in range(DB) for dc in range(NDC)]

        # final drain, staggered: hb0-2 of a few pairs run ahead so the
        # last head's normalize chain is hidden behind PE work
        y_open = []
        k = 0
        drain = prev_wo_parts[-len(prev_wo):] if prev_wo else []
        prev_wo = []
        for head, finish in drain:
            y_open.append((finish, head(HL - 1)))
            if len(y_open) > 3:
                f, yp = y_open.pop(0)
                f(yp)
        while y_open:
            f, yp = y_open.pop(0)
            f(yp)
    nc.compile()
    return nc


def host_inputs(hidden_states, wq, wk, wv, wo, position_ids,
                S=2048, D=4096, HL=4, QT=512, ncores=NCORES):
    """Shard + preprocess inputs -> per-core in_maps."""
    import ml_dtypes
    cast = lambda a: np.ascontiguousarray(a).astype(ml_dtypes.bfloat16)
    DB = QT // 128
    hs = np.asarray(hidden_states, np.float32)[0]
    hsT = cast(hs.T)  # [D, S]

    pos = np.asarray(position_ids)[0].astype(np.float32)
    inv_freq = (1.0 / (10000.0 ** (np.arange(0, 128, 2, dtype=np.float32) / 128.0)))
    freqs = pos[:, None] * inv_freq[None, :]          # [S, 64]
    emb = np.concatenate([freqs, freqs], axis=1)      # [S, 128]
    cosT = np.ascontiguousarray(np.cos(emb).T).astype(np.float32)
    sinT = np.ascontiguousarray(np.sin(emb).T).astype(np.float32)

    kk = np.arange(128)[:, None]
    qq = np.arange(128)[None, :]
    mks = (kk <= qq).astype(ml_dtypes.bfloat16)  # [128,128] triangular

    rotT = np.zeros((128, 128), np.float32)
    idx = np.arange(64)
    rotT[idx, idx + 64] = 1.0
    rotT[idx + 64, idx] = -1.0
    rotT = cast(rotT)

    wq = np.asarray(wq, np.float32)
    wk = np.asarray(wk, np.float32)
    wv = np.asarray(wv, np.float32)
    wo = np.asarray(wo, np.float32)

    in_maps = []
    qh = HL * 128
    for c in range(ncores):
        wqT_c = cast(wq[c * qh:(c + 1) * qh, :].T)
        wkT_c = cast(wk[c * 128:(c + 1) * 128, :].T)
        wvT_c = cast(wv[c * 128:(c + 1) * 128, :].T)
        woT_c = cast(wo[:, c * qh:(c + 1) * qh].T)
        in_maps.append({
            "hsT": hsT, "wqT": wqT_c, "wkT": wkT_c, "wvT": wvT_c,
            "woT": woT_c, "cosT": cosT, "sinT": sinT, "mks": mks,
            "rotT": rotT,
        })
    return in_maps


_NC_CACHE = {}
COMPUTE = "bf16"


def kernel(hidden_states, wq, wk, wv, wo, position_ids):
    B, S, D = hidden_states.shape
    in_maps = host_inputs(hidden_states, wq, wk, wv, wo, position_ids,
                          S=S, D=D)
    key = (S, D, COMPUTE)
    if key not in _NC_CACHE:
        _NC_CACHE[key] = build_nc(S=S, D=D)
    nc = _NC_CACHE[key]
    res = run_bass_kernel_spmd(nc, in_maps, core_ids=list(range(NCORES)),
                               trace=False)
    y = np.zeros((S, D), np.float32)
    for c in range(NCORES):
        y += res.results[c]["y"].astype(np.float32)
    return y[None]
